# revision 8
# baseline (speedup 1.0000x reference)
"""Chunked local attention (B=4, S=8192, D=1024, H=16, Dh=64, C=256, W=64)
on 8 Trainium2 NeuronCores.

Sharding: data-parallel over the 128 (batch x chunk) units -> 16 chunks/core.
The host passes each core its x shard pre-transposed to bf16 [D, tok]
(the projections contract over d, so x must be d-partitioned in SBUF);
weights are replicated bf16.

All matmuls are single-pass bf16 with fp32 PSUM accumulation (end-to-end
rel err ~4e-3 vs the fp32 reference, well inside the 2e-2 gate; 3x fewer
PE cycles than the hi/lo 3-pass scheme this replaces).

PE weight-reload latency (~450 ns per cold lhsT switch, measured: 191 ns
same-lhsT vs 638 ns alternating-lhsT for N=512 matmuls) dominates over
streaming, so the loop structure maximizes columns streamed per stationary
load: chunks are processed in PAIRS (q/k stream 512 token columns per
weight tile) and the v/y projections interleave their two 512-wide dout
halves under a single stationary load (consecutive same-lhsT matmuls run
at full stream speed).

Per-core dataflow (per chunk pair):
  qT/kT = Wq^T/Wk^T @ xT     (lhsT=W native, rhs=xT[512])  [dout, tok]
  v     = xT^T @ Wv          (lhsT=xT, rhs=Wv, nn-interleaved)
  per chunk in pair:
    sT_h  = kT_h^T-x-qT_h    (lhsT=kT_h, rhs=qT_h)         [j, i] per head
    p_h   = bf16(exp(0.125*sT_h)) * bandmask               [j, i]
    oU|den= p_h^T @ [v_h|1]  (lhsT=p_h, rhs=v_aug)  fp32   [i, 65] per head
            (ones column on v gives the softmax denominator for free)
    oN    = oU * (1/den)     fp32, per-head bcast, fused in PSUM copy
    oT    = PE-transpose(oN) fp32 in, bf16 out via scalar copy
    y     = oT^T @ Wo        (lhsT=oT, rhs=Wo, nn-interleaved)
"""

from contextlib import ExitStack

import ml_dtypes
import numpy as np

import concourse.bass as bass
import concourse.mybir as mybir
import concourse.tile as tile
from concourse import bacc
from concourse.bass_utils import run_bass_kernel_spmd
from concourse.masks import make_identity

B, S, D = 4, 8192, 1024
H, DH, C, W = 16, 64, 256, 64
NCORES = 8
NCHUNKS_TOTAL = B * (S // C)      # 128
CPC = NCHUNKS_TOTAL // NCORES     # 16 chunks per core
TPC = CPC * C                     # 4096 tokens per core
F32 = mybir.dt.float32
BF16 = mybir.dt.bfloat16
KT = D // 128                     # 8 k-tiles over the contraction dim
BF = ml_dtypes.bfloat16
WNAMES = ("wq", "wk", "wv", "wo")
C2 = 2 * C                        # tokens per chunk pair


def _band_mask_np():
    # maskT[jt, jj, i] = 1 iff j <= i <= j+W, j = jt*128+jj  (layout [j, i])
    j = np.arange(C)[:, None]
    i = np.arange(C)[None, :]
    m = ((j <= i) & (i <= j + W)).astype(BF)
    return np.ascontiguousarray(m.reshape(2, 128, C))


def _emit(ctx, tc, io, n_chunks):
    nc = tc.nc
    xn_d, w_d, mask_d, y_d = io
    assert n_chunks % 2 == 0

    singles = ctx.enter_context(tc.tile_pool(name="singles", bufs=1))
    xtpool = ctx.enter_context(tc.tile_pool(name="xtpool", bufs=2))
    qkpool = ctx.enter_context(tc.tile_pool(name="qkpool", bufs=2))
    vpool = ctx.enter_context(tc.tile_pool(name="vpool", bufs=2))
    ppool = ctx.enter_context(tc.tile_pool(name="ppool", bufs=8))
    opool = ctx.enter_context(tc.tile_pool(name="opool", bufs=2))
    otpool = ctx.enter_context(tc.tile_pool(name="otpool", bufs=2))
    ypool = ctx.enter_context(tc.tile_pool(name="ypool", bufs=2))
    dnpool = ctx.enter_context(tc.tile_pool(name="dnpool", bufs=4))

    ps256 = ctx.enter_context(tc.tile_pool(name="ps256", bufs=2, space="PSUM"))
    ps512 = ctx.enter_context(tc.tile_pool(name="ps512", bufs=4, space="PSUM"))
    psbig = ctx.enter_context(tc.tile_pool(name="psbig", bufs=2, space="PSUM"))

    # --- constants / weights resident in SBUF (bf16) ---
    # Weights/mask load on the GPSIMD (SWDGE) queues so the per-chunk x/y
    # traffic on the HWDGE queues is not stuck behind 8MB of weights at
    # kernel start.
    w_sb = {}
    for wn in WNAMES:
        kts = []
        for kt in range(KT):
            t = singles.tile([128, D], BF16, tag=f"{wn}{kt}", name=f"{wn}{kt}")
            nc.gpsimd.dma_start(
                out=t, in_=w_d[wn].ap()[kt * 128:(kt + 1) * 128, :])
            kts.append(t)
        w_sb[wn] = kts
    mask_sb = singles.tile([128, 2, C], BF16, tag="mask")
    nc.gpsimd.dma_start(out=mask_sb, in_=mask_d.ap().rearrange("jt p i -> p jt i"))
    ident = singles.tile([128, 128], F32, tag="ident")
    make_identity(nc, ident)

    for cp in range(n_chunks // 2):
        tok0 = ((2 * cp) % CPC) * C   # first token of the pair (512 tokens)

        # --- load xT pair: [128, KT, 512] bf16 (pre-transposed on host) ---
        xT = xtpool.tile([128, KT, C2], BF16, tag="xT")
        nc.sync.dma_start(
            out=xT,
            in_=xn_d.ap()[:, tok0:tok0 + C2].rearrange("(kt p) t -> p kt t", p=128))

        # --- qT, kT projections -> bf16 [dout-par, m, tok(512)] ---
        # one stationary W tile streams all 512 pair tokens
        qk = {}
        for wn in ("wq", "wk"):
            t = qkpool.tile([128, KT, C2], BF16, tag=wn + "T", name=wn + "T")
            for m in range(KT):
                ps = ps512.tile([128, C2], F32, tag="ps512")
                msl = slice(m * 128, (m + 1) * 128)
                for kt in range(KT):
                    nc.tensor.matmul(ps, w_sb[wn][kt][:, msl], xT[:, kt, :],
                                     start=(kt == 0), stop=(kt == KT - 1))
                nc.scalar.copy(out=t[:, m, :], in_=ps)
            qk[wn] = t
        qT, kT = qk["wq"], qk["wk"]

        # --- v projection: v_sb[j-par, jt(4), head, 65] bf16, ones column
        # gives the softmax denominator for free.  The two 512-wide dout
        # halves interleave under one stationary xT tile (same-lhsT pairs).
        v_sb = vpool.tile([128, 4, H, DH + 1], BF16, tag="v")
        nc.vector.memset(v_sb[:, :, :, DH:], 1.0)
        for jt in range(4):
            jsl = slice(jt * 128, (jt + 1) * 128)
            pss = [ps512.tile([128, 512], F32, tag="ps512",
                              name=f"v{cp}_{jt}_{nn}") for nn in range(2)]
            for kt in range(KT):
                for nn in range(2):
                    nc.tensor.matmul(pss[nn], xT[:, kt, jsl],
                                     w_sb["wv"][kt][:, nn * 512:(nn + 1) * 512],
                                     start=(kt == 0), stop=(kt == KT - 1))
            for nn in range(2):
                nc.vector.tensor_copy(
                    out=v_sb[:, jt, nn * 8:(nn + 1) * 8, :DH],
                    in_=pss[nn].rearrange("p (h d) -> p h d", h=8))

        for ci in range(2):
            cb = ci * C              # qT/kT column base for this chunk
            jb = ci * 2              # v_sb j-tile base
            # --- attention, 4 heads (one PSUM bank of [128,4,65]) at a time ---
            oN = [opool.tile([128, D], F32, tag="oN", name=f"oN{cp}_{ci}_{i}")
                  for i in range(2)]
            for qt in range(4):
                o_ps = [psbig.tile([128, 4, 2 * DH], F32, tag="obig",
                                   name=f"o_ps{cp}_{ci}_{qt}_{i}")
                        for i in range(2)]
                # head pairs (2*hm, 2*hm+1) sit on partitions 0-63 / 64-127;
                # their K=64 score matmuls use disjoint PE row groups and are
                # emitted interleaved so they run concurrently in the array.
                for pr in range(2):
                    hm = qt * 2 + pr
                    h0, h1 = 2 * hm, 2 * hm + 1
                    lo64, hi64 = slice(0, 64), slice(64, 128)
                    # Band (j <= i <= j+64): j-tile0 feeds i in [0,192) only,
                    # j-tile1 feeds i in [128,256) only.  Compute scores, exp
                    # and mask only on those column bands.
                    bsl = (slice(0, 192), slice(128, C))
                    probs = {}   # (head_in_pair, jt) -> tile
                    for jt in range(2):
                        jsl = slice(cb + jt * 128, cb + (jt + 1) * 128)
                        isl = bsl[jt]
                        csl = slice(cb + isl.start, cb + isl.stop)
                        s0 = ps256.tile([128, C], F32, tag="ps256",
                                        name=f"s0_{cp}_{ci}_{hm}_{jt}")
                        s1 = ps256.tile([128, C], F32, tag="ps256",
                                        name=f"s1_{cp}_{ci}_{hm}_{jt}")
                        nc.tensor.matmul(s0[:, isl], kT[lo64, hm, jsl],
                                         qT[lo64, hm, csl], start=True, stop=True)
                        nc.tensor.matmul(s1[:, isl], kT[hi64, hm, jsl],
                                         qT[hi64, hm, csl], start=True, stop=True)
                        for hp, s_ps in ((0, s0), (1, s1)):
                            p_sb = ppool.tile([128, C], BF16, tag="probs",
                                              name=f"p_{cp}_{ci}_{hm}_{jt}_{hp}")
                            nc.scalar.activation(
                                out=p_sb[:, isl], in_=s_ps[:, isl],
                                func=mybir.ActivationFunctionType.Exp, scale=0.125)
                            nc.vector.tensor_mul(p_sb[:, isl], p_sb[:, isl],
                                                 mask_sb[:, jt, isl])
                            probs[(hp, jt)] = p_sb
                    # PV (+den via ones column). i-tile0: j-tile0 only.
                    # i-tile1: full-M j-tile1 matmul first (start=True covers
                    # all 128 partitions), then the 64-wide j-tile0 partial
                    # (i in [128,192)) accumulates into partitions 0:64.
                    for hp, h in ((0, h0), (1, h1)):
                        hq = h - qt * 4
                        nc.tensor.matmul(
                            o_ps[0][:, hq, :DH + 1],
                            probs[(hp, 0)][:, 0:128],
                            v_sb[:, jb + 0, h, :],
                            start=True, stop=True)
                        nc.tensor.matmul(
                            o_ps[1][64:128, hq, :DH + 1],
                            probs[(hp, 1)][:, 192:C],
                            v_sb[:, jb + 1, h, :],
                            start=True, stop=True)
                        nc.tensor.matmul(
                            o_ps[1][0:64, hq, :DH + 1],
                            probs[(hp, 1)][:, 128:192],
                            v_sb[:, jb + 1, h, :],
                            start=True, stop=False)
                        nc.tensor.matmul(
                            o_ps[1][0:64, hq, :DH + 1],
                            probs[(hp, 0)][:, 128:192],
                            v_sb[:, jb + 0, h, :],
                            start=False, stop=True)
                # normalize this quarter: oN = oU * (1/den), fused in PSUM copy
                for it in range(2):
                    denr = dnpool.tile([128, 4], F32, tag="denr")
                    nc.vector.reciprocal(out=denr, in_=o_ps[it][:, :, DH])
                    denr_bc = bass.AP(
                        tensor=denr.tensor, offset=denr.offset,
                        ap=[denr.ap[0], denr.ap[1], [0, DH]])
                    nc.vector.tensor_mul(
                        oN[it][:, qt * 256:(qt + 1) * 256]
                        .rearrange("p (h d) -> p h d", h=4),
                        o_ps[it][:, :, :DH],
                        denr_bc)

            # --- transpose oN -> oT [dout-par, dt, i] (fp32 in, bf16 out) ---
            oT = otpool.tile([128, KT, C], BF16, tag="oT")
            for dt in range(KT):
                ps = ps256.tile([128, C], F32, tag="ps256",
                                name=f"tp_{cp}_{ci}_{dt}")
                for it in range(2):
                    nc.tensor.transpose(ps[:, it * 128:(it + 1) * 128],
                                        oN[it][:, dt * 128:(dt + 1) * 128], ident)
                nc.scalar.copy(out=oT[:, dt, :], in_=ps)

            # --- y projection + store; nn halves interleaved under one
            # stationary oT tile (same-lhsT pairs) ---
            for it in range(2):
                isl = slice(it * 128, (it + 1) * 128)
                y_sb = ypool.tile([128, D], F32, tag="y")
                pss = [ps512.tile([128, 512], F32, tag="ps512",
                                  name=f"y{cp}_{ci}_{it}_{nn}") for nn in range(2)]
                for dt in range(KT):
                    for nn in range(2):
                        nc.tensor.matmul(pss[nn], oT[:, dt, isl],
                                         w_sb["wo"][dt][:, nn * 512:(nn + 1) * 512],
                                         start=(dt == 0), stop=(dt == KT - 1))
                for nn in range(2):
                    nc.vector.tensor_copy(out=y_sb[:, nn * 512:(nn + 1) * 512],
                                          in_=pss[nn])
                nc.sync.dma_start(
                    out=y_d.ap()[tok0 + cb + it * 128:
                                 tok0 + cb + (it + 1) * 128, :],
                    in_=y_sb)


def build(n_chunks=CPC, n_cores=NCORES):
    nc = bacc.Bacc("TRN2", target_bir_lowering=False, debug=False,
                   num_devices=n_cores)
    xn_d = nc.dram_tensor("xth", [D, TPC], BF16, kind="ExternalInput")
    w_d = {}
    for wn in WNAMES:
        w_d[wn] = nc.dram_tensor(wn, [D, D], BF16, kind="ExternalInput")
    mask_d = nc.dram_tensor("maskt", [2, 128, C], BF16, kind="ExternalInput")
    y_d = nc.dram_tensor("y", [TPC, D], F32, kind="ExternalOutput")
    io = (xn_d, w_d, mask_d, y_d)
    with tile.TileContext(nc) as tc, ExitStack() as ctx:
        _emit(ctx, tc, io, n_chunks)
    nc.compile()
    return nc


def make_in_maps(x, Wq, Wk, Wv, Wo):
    xc = np.asarray(x, np.float32).reshape(NCHUNKS_TOTAL, C, D)
    mask = _band_mask_np()
    wmap = {wn: np.ascontiguousarray(np.asarray(w, np.float32).astype(BF))
            for wn, w in zip(WNAMES, (Wq, Wk, Wv, Wo))}
    in_maps = []
    for s in range(NCORES):
        shard = xc[s * CPC:(s + 1) * CPC].reshape(TPC, D)
        xT = np.ascontiguousarray(shard.T).astype(BF)
        in_maps.append({"xth": xT, "maskt": mask, **wmap})
    return in_maps


_NC_CACHE = {}


def kernel(x, Wq, Wk, Wv, Wo):
    if "nc" not in _NC_CACHE:
        _NC_CACHE["nc"] = build()
    nc = _NC_CACHE["nc"]
    in_maps = make_in_maps(x, Wq, Wk, Wv, Wo)
    res = run_bass_kernel_spmd(nc, in_maps, core_ids=list(range(NCORES)))
    out = np.concatenate([res.results[s]["y"] for s in range(NCORES)], axis=0)
    return out.reshape(B, S, D).astype(np.float32)
